# revision 26
# baseline (speedup 1.0000x reference)
"""Trainium2 Bass kernel for nn_DA3CrossFrameRKDDistanceLoss.

Math (reference semantics):
  ref rows (teacher/student frame 0, ref_perm subsample), extra = teacher
  frames [1,3,5,7] concat -> [4096, D].  Cosine top-4 neighbours of each ref
  row inside extra;  KL(softmax(diff_t) || softmax(diff_s)) per row with
  diff pairs (d1: ref-shared, d2: ref-simhigh, d3: shared-simhigh), smooth-L1
  (beta=0.5) of each KL, averaged per branch and summed.

  Per unit, with a = diff_t, b = diff_s over D:
    kl = N/Sa - ln Sa + ln Sb,  Sa = sum exp(a), Sb = sum exp(b),
    N = sum exp(a)*(a-b).
  d2/d3 factor through exp(x - sh) = exp(x)*exp(-sh): with q_j = exp(-sh_j),
    Sa = <exp(x_t), q_j>, Sb = <exp(x_s), q_j>, N = <exp(x_t)*(x_t-x_s), q_j>,
  so each sum is ONE bf16 elementwise product (DVE/Pool) of tensors that are
  PRECOMPUTED once, plus a reduction.

Layout trick: all d2/d3 tensors live TRANSPOSED [d_lo=128 part, kt, ref] so
the reductions over D are 8 accumulating PE matmuls against a ones-column
(contraction over partitions = free on the PE; the expensive free-axis DVE
reduce is never used).  sim_high rows are gathered naturally [ref, D] from a
bf16 copy of extra, exp(-x) on ACT, then PE-transposed (8 tiles/j) into PSUM.

Top-4 search: bf16 extT stream (verified: 22/4096 top-4 flips, 1.4e-5 final
loss shift), per-chunk Max/MaxIndex on DVE hidden under the extT DMA, global
top-4 via Max over the 64 chunk-candidates + iota/is_equal index extraction
(no 4.3us full-sim MaxIndex on the critical path).

Sharding: 8 cores = (batch b in 0..3) x (half h of the 256 ref rows), host
sums the per-core smooth-L1 outputs (scalar "all-reduce").

Build quirks for this container's walrus: at most ONE sync-wait encodes per
compute instruction -> _split_waits() moves extras onto same-engine Drain
carriers; fused DVE reduce ops fail codegen, which is fine since reductions
ride the PE here.
"""

import os
import sys

import numpy as np

for _p in ("/opt/trn_rl_repo", "/root/.axon_site/_ro/trn_rl_repo"):
    # later inserts go to the front: prefer the axon-site copy when present
    if os.path.isdir(_p) and _p not in sys.path:
        sys.path.insert(0, _p)

import concourse.bass as bass
import concourse.tile as tile
from concourse import mybir
from concourse.bass_utils import run_bass_kernel_spmd

F32 = mybir.dt.float32
BF16 = mybir.dt.bfloat16
U16 = mybir.dt.uint16
I32 = mybir.dt.int32

B = 4
P = 1024
D = 1024
NUM_REF = 256
TOPK = 4
NREF_CORE = 128
NEXTRA = 4 * P          # 4096
EXTRA_FRAMES = (1, 3, 5, 7)
SHARED_T = (2, 4, 6)
SHARED_S = (1, 2, 3)
NFRAMES = 3
KT = D // 128           # 8 contraction tiles
N_CHUNK = 8
CHUNK = NEXTRA // N_CHUNK   # 512
# units: 3 d1 + 4 d2 + 12 d3 = 19; psum sum columns = unit*3 + {Sa,Sb,N}
N_UNITS = 19

ALU = mybir.AluOpType
ACTF = mybir.ActivationFunctionType

_BUILT = None
POOL_MULS = True
EARLY_WB = True
J0_PSUM = True


def _split_waits(nc):
    """Walrus in this container encodes at most one sync-wait per compute
    instruction. Split extras onto same-engine Drain carriers placed just
    before (engines execute in program order, so semantics are identical)."""
    ctr = [0]

    def process(block):
        new = []
        for inst in block.instructions:
            si = inst.sync_info
            waits = list(si.on_wait) if si is not None and si.on_wait else []
            if len(waits) > 1:
                drains = []
                for w in waits[:-1]:
                    ctr[0] += 1
                    drains.append(mybir.InstDrain(
                        name=f"waitnop-{ctr[0]}",
                        engine=inst.engine,
                        ins=[],
                        outs=[],
                        sync_info=mybir.SyncInfo(on_wait=[w], on_update=[]),
                    ))
                inst.sync_info = mybir.SyncInfo(
                    on_wait=[waits[-1]], on_update=list(si.on_update or [])
                )
                # never separate an Ldweights from its Matmult: walrus fuses
                # the adjacent pair, and the weights load must stay behind
                # every wait that guards it
                pos = len(new)
                if (inst.opcode == "Matmult" and new
                        and new[-1].opcode == "Ldweights"
                        and new[-1].engine == inst.engine):
                    pos -= 1
                new[pos:pos] = drains
            new.append(inst)
        block.instructions = new
        for b in getattr(block, "blocks", []) or []:
            process(b)

    for b in nc.m.functions[0].blocks:
        process(b)


def _build_module():
    """Trace the per-core Bass program (identical on all 8 cores)."""
    nc = bass.Bass()

    # pre-swizzled transposed layouts (see make_in_maps)
    refTb = nc.declare_dram_parameter("refTb", [128, KT, NREF_CORE], BF16,
                                      isOutput=False)
    extTb = nc.declare_dram_parameter("extTb", [128, KT, NEXTRA], BF16,
                                      isOutput=False)
    # smallT[t]: 0 rt, 1 rs, 2-4 st_f, 5-7 ss_f  (transposed [d_lo, kt, ref])
    smallT = nc.declare_dram_parameter("smallT", [128, 8, KT, NREF_CORE],
                                       BF16, isOutput=False)
    extnatb = nc.declare_dram_parameter("extnatb", [NEXTRA, D], BF16,
                                        isOutput=False)
    ident_d = nc.declare_dram_parameter("ident", [128, 128], BF16,
                                        isOutput=False)
    # iotaf[:, 0:N_CHUNK * 8] = 0..63 ; iotaf[:, 64:64 + N_CHUNK * 8] = chunk offsets c*CHUNK per col
    iotaf_d = nc.declare_dram_parameter("iotaf", [128, 128], F32,
                                        isOutput=False)
    sums_d = nc.declare_dram_parameter("sums", [NREF_CORE, 64], F32,
                                       isOutput=True)

    with tile.TileContext(nc) as tc:
        with (
            tc.tile_pool(name="singles", bufs=1) as singles,
            tc.tile_pool(name="ext", bufs=4) as ext,
            tc.tile_pool(name="psc", bufs=4, space="PSUM") as psc,
            tc.tile_pool(name="psq", bufs=2, space="PSUM") as psq,
            tc.tile_pool(name="pss", bufs=1, space="PSUM") as pss,
            tc.tile_pool(name="prods", bufs=10) as prods,
            tc.tile_pool(name="prodsp", bufs=3) as prodsp,
        ):
            dma = nc.sync.dma_start

            # ---- resident tiles -------------------------------------------
            ident = singles.tile([128, 128], BF16)
            iotaf = singles.tile([128, 128], F32)
            onescol = singles.tile([128, 8], BF16)
            refT_sb = singles.tile([128, KT, NREF_CORE], BF16)
            sm = singles.tile([128, 8, KT, NREF_CORE], BF16)
            # DMA order drives everything: refTb + first extT chunk first so
            # PE starts ~4us in; smalls next (rt/rs slice first so the exp/
            # diff precompute starts early); tiny consts after; chunks 1-7.
            dma(out=refT_sb, in_=refTb.ap())
            dma(out=sm[:, 0:2], in_=smallT.ap()[:, 0:2])
            nc.gpsimd.memset(onescol, 1.0)

            rtT = sm[:, 0]
            rsT = sm[:, 1]
            stT = [sm[:, 2 + f] for f in range(NFRAMES)]
            ssT = [sm[:, 5 + f] for f in range(NFRAMES)]

            exps = singles.tile([128, 8, KT, NREF_CORE], BF16)
            eRtT = exps[:, 0]
            eRsT = exps[:, 1]
            eStT = [exps[:, 2 + f] for f in range(NFRAMES)]
            eSsT = [exps[:, 5 + f] for f in range(NFRAMES)]

            cs = singles.tile([128, 4, KT, NREF_CORE], BF16)   # c2, c3_f
            c2T = cs[:, 0]
            c3T = [cs[:, 1 + f] for f in range(NFRAMES)]
            us = singles.tile([128, 4, KT, NREF_CORE], BF16)   # u2, u3_f
            u2T = us[:, 0]
            u3T = [us[:, 1 + f] for f in range(NFRAMES)]
            c1T = singles.tile([128, NFRAMES, KT, NREF_CORE], BF16)
            d1ab = singles.tile([128, 6, KT, NREF_CORE], BF16)
            d1e = singles.tile([128, 6, KT, NREF_CORE], BF16)
            d1n = singles.tile([128, NFRAMES, KT, NREF_CORE], BF16)

            cand = singles.tile([128, N_CHUNK * 8], F32)
            ci_all = singles.tile([128, N_CHUNK * 8], U16)
            candi = singles.tile([128, N_CHUNK * 8], F32)
            topv = singles.tile([128, 8], F32)
            pos8 = singles.tile([128, 8], U16)
            posf = singles.tile([128, 8], F32)
            idxf = singles.tile([128, TOPK], F32)
            idx32 = singles.tile([128, TOPK], I32)
            sh = singles.tile([128, TOPK, D], BF16)
            q = singles.tile([128, TOPK, D], BF16)

            # all 57 sums accumulate here: col = unit*3 + {0 Sa, 1 Sb, 2 N}
            sums = pss.tile([128, 57], F32)

            def onesmm(col, prod):
                for k in range(KT):
                    nc.tensor.matmul(
                        sums[:, col:col + 1], lhsT=prod[:, k, :],
                        rhs=onescol[:, :1], start=(k == 0), stop=(k == KT - 1),
                        skip_group_check=True,
                    )

            # ---- phase 0: stream extT chunks; matmul + per-chunk top8 -----
            # interleave gather-independent DVE/ACT/Pool work between chunks
            # so no engine starves while DMA streams.
            def emit_precompute():
                # rt/rs-only work first (arrives in the first smallT slice);
                # DVE keeps only the ea-chain ops (chunk top8 work saturates
                # it); everything else rides the otherwise-idle Pool.
                yield lambda: nc.scalar.activation(exps[:, 0], sm[:, 0],
                                                   ACTF.Exp)
                yield lambda: nc.scalar.activation(exps[:, 1], sm[:, 1],
                                                   ACTF.Exp)
                yield lambda: nc.vector.tensor_sub(c2T, rtT, rsT)
                yield lambda: nc.vector.tensor_mul(u2T, eRtT, c2T)
                for t in range(2, 8):
                    yield lambda t=t: nc.scalar.activation(
                        exps[:, t], sm[:, t], ACTF.Exp)
                for f in range(NFRAMES):
                    yield lambda f=f: nc.gpsimd.tensor_sub(
                        c3T[f], stT[f], ssT[f])
                # d1 units: a/b subs, exps, c1, N-prod, ones-mms
                for f in range(NFRAMES):
                    a = d1ab[:, f]
                    bb = d1ab[:, 3 + f]
                    ea = d1e[:, f]
                    eb = d1e[:, 3 + f]
                    yield lambda f=f, a=a: nc.vector.tensor_sub(
                        a, rtT, stT[f])
                    yield lambda f=f, bb=bb: nc.gpsimd.tensor_sub(
                        bb, rsT, ssT[f])
                    yield lambda a=a, ea=ea: nc.scalar.activation(
                        ea, a, ACTF.Exp)
                    yield lambda bb=bb, eb=eb: nc.scalar.activation(
                        eb, bb, ACTF.Exp)
                    yield lambda f=f: nc.vector.tensor_sub(
                        c1T[:, f], c2T, c3T[f])
                    yield lambda f=f: nc.gpsimd.tensor_mul(
                        u3T[f], eStT[f], c3T[f])
                    yield lambda f=f, ea=ea: nc.vector.tensor_mul(
                        d1n[:, f], ea, c1T[:, f])
                    yield lambda f=f, ea=ea: onesmm(3 * f + 0, ea)
                    yield lambda f=f, eb=eb: onesmm(3 * f + 1, eb)
                    yield lambda f=f: onesmm(3 * f + 2, d1n[:, f])

            pre = emit_precompute()

            def drain_pre(n):
                for _ in range(n):
                    try:
                        next(pre)()
                    except StopIteration:
                        return

            for c in range(N_CHUNK):
                et = ext.tile([128, KT, CHUNK], BF16, tag="et")
                dma(out=et, in_=extTb.ap()[:, :, c * CHUNK:(c + 1) * CHUNK])
                if c == 0:
                    # rest of the smalls ride the queue right after chunk 0;
                    # tiny consts go AFTER the chunk stream (needed only at
                    # the boundary / for the d1 ones-matmuls)
                    dma(out=sm[:, 2:8], in_=smallT.ap()[:, 2:8])
                pt = psc.tile([128, CHUNK], F32, tag="pt")
                for k in range(KT):
                    nc.tensor.matmul(
                        pt, lhsT=refT_sb[:, k, :], rhs=et[:, k, :],
                        start=(k == 0), stop=(k == KT - 1),
                    )
                nc.vector.max(cand[:, c * 8:(c + 1) * 8], pt)
                nc.vector.max_index(ci_all[:, c * 8:(c + 1) * 8],
                                    cand[:, c * 8:(c + 1) * 8], pt)
                # ~5 precompute ops per chunk gap keeps DVE/ACT/Pool fed
                drain_pre(5)
            dma(out=iotaf, in_=iotaf_d.ap())
            dma(out=ident, in_=ident_d.ap())
            drain_pre(100)

            # ---- boundary: global top4 + index extraction + gather --------
            nc.vector.tensor_copy(candi, ci_all)
            nc.vector.tensor_add(candi, candi, iotaf[:, 64:64 + N_CHUNK * 8])
            nc.vector.max(topv, cand)
            nc.vector.max_index(pos8, topv, cand)
            nc.vector.tensor_copy(posf, pos8)
            eq = singles.tile([128, TOPK, N_CHUNK * 8], F32)
            for j in range(TOPK):
                nc.vector.tensor_scalar(
                    eq[:, j], iotaf[:, 0:N_CHUNK * 8], posf[:, j:j + 1], None,
                    op0=ALU.is_equal,
                )
                nc.vector.tensor_mul(eq[:, j], eq[:, j], candi)
                nc.vector.reduce_sum(
                    idxf[:, j:j + 1], eq[:, j], axis=mybir.AxisListType.X)
            nc.vector.tensor_copy(idx32, idxf)

            # per-j SWDGE gathers ([128,1] offsets only: wider offset APs
            # return wrong data on this runtime); j=0 first so its chain
            # starts while the rest prep/transfer
            for j in range(TOPK):
                nc.gpsimd.indirect_dma_start(
                    out=sh[:, j, :],
                    out_offset=None,
                    in_=extnatb.ap(),
                    in_offset=bass.IndirectOffsetOnAxis(
                        ap=idx32[:, j:j + 1], axis=0),
                )

            # ---- phase 1: q_j, transposes, 48 product sums ----------------
            # products: DVE 10/j, Pool 2/j (separate tile tags so the slow
            # Pool muls never block DVE's buffer rotation); the ones-matmuls
            # ride the otherwise idle PE; muls read qT straight from PSUM.
            qTs = singles.tile([128, TOPK, KT, 128], BF16)
            for j in range(TOPK):
                nc.scalar.activation(q[:, j], sh[:, j], ACTF.Exp, scale=-1.0)
                qT = psq.tile([128, KT, 128], BF16, tag="qT", name=f"qT{j}")
                for k in range(KT):
                    nc.tensor.transpose(
                        qT[:, k, :], q[:, j, k * 128:(k + 1) * 128], ident)
                nc.scalar.copy(qTs[:, j], qT)
                qTj_dve = qT if (J0_PSUM and j == 0) else qTs[:, j]
                qTj_pool = qTs[:, j]

                # cols grouped by completion order: group j owns 9+12j..20+12j
                base = 9 + 12 * j
                pool_set = (1, 7) if j % 2 == 0 else (1, 4, 10)
                srcs = [
                    (eRtT, base + 0),
                    (eRsT, base + 1),
                    (u2T, base + 2),
                ]
                for f in range(NFRAMES):
                    srcs += [
                        (eStT[f], base + 3 + 3 * f + 0),
                        (eSsT[f], base + 3 + 3 * f + 1),
                        (u3T[f], base + 3 + 3 * f + 2),
                    ]
                for i, (src, col) in enumerate(srcs):
                    if POOL_MULS and i in pool_set:
                        prod = prodsp.tile([128, KT, 128], BF16, tag="prodp")
                        nc.gpsimd.tensor_mul(prod, src, qTj_pool)
                    else:
                        prod = prods.tile([128, KT, 128], BF16, tag="prod")
                        nc.vector.tensor_mul(prod, src, qTj_dve)
                    onesmm(col, prod)
                if EARLY_WB and j == 2:
                    # early writeback of everything finished by now; the
                    # final DMA then only carries j=3's 12 columns
                    sums_sb1 = singles.tile([128, 45], F32)
                    nc.scalar.copy(sums_sb1, sums[:, 0:45])
                    dma(out=sums_d.ap()[:, 0:45], in_=sums_sb1)

            # raw sums out; host does the tiny kl/huber/mean tail in fp64
            sums_sb2 = singles.tile([128, 12], F32)
            nc.scalar.copy(sums_sb2, sums[:, 45:57])
            dma(out=sums_d.ap()[:, 45:57], in_=sums_sb2)

    _split_waits(nc)
    return nc


def get_module():
    global _BUILT
    if _BUILT is None:
        _BUILT = _build_module()
    return _BUILT


def make_in_maps(teacher_feats, student_feats, ref_perm, shared_perm):
    """Host-side sharding: slice/normalize/transpose the per-core inputs."""
    import ml_dtypes

    tf = np.ascontiguousarray(np.asarray(teacher_feats, dtype=np.float32))
    sf = np.ascontiguousarray(np.asarray(student_feats, dtype=np.float32))
    rp = np.asarray(ref_perm, dtype=np.int64)
    sp = np.asarray(shared_perm, dtype=np.int64)[:NUM_REF]

    ident = np.eye(128, dtype=ml_dtypes.bfloat16)
    iotaf = np.zeros((128, 128), dtype=np.float32)
    iotaf[:, 0:N_CHUNK * 8] = np.arange(64, dtype=np.float32)[None, :]
    iotaf[:, 64:64 + N_CHUNK * 8] = np.repeat(
        np.arange(N_CHUNK, dtype=np.float32) * CHUNK, 8)[None, :]

    def tr(x):
        # [rows, D] f32 -> [128 d_lo, KT, rows] bf16 (d = kt*128 + d_lo)
        t = x.T.reshape(KT, 128, x.shape[0]).transpose(1, 0, 2)
        return np.ascontiguousarray(t.astype(ml_dtypes.bfloat16))

    in_maps = []
    for b in range(B):
        extra = np.ascontiguousarray(
            tf[b, list(EXTRA_FRAMES)].reshape(NEXTRA, D))
        en = np.maximum(
            np.sqrt((extra ** 2).sum(axis=1, keepdims=True)), 1e-12)
        extn = extra / en
        extTb = tr(extn)                       # [128, KT, 4096]
        extnatb = np.ascontiguousarray(extra.astype(ml_dtypes.bfloat16))
        ref_t = tf[b, 0][rp]
        ref_s = sf[b, 0][rp]
        rn = ref_t / np.maximum(
            np.sqrt((ref_t ** 2).sum(axis=1, keepdims=True)), 1e-12)
        st_all = [tf[b, t][sp] for t in SHARED_T]
        ss_all = [sf[b, s][sp] for s in SHARED_S]
        for h in range(2):
            sl = slice(h * NREF_CORE, (h + 1) * NREF_CORE)
            small = np.stack(
                [tr(ref_t[sl]), tr(ref_s[sl])]
                + [tr(x[sl]) for x in st_all]
                + [tr(x[sl]) for x in ss_all], axis=1)   # [128, 8, KT, 128]
            in_maps.append(
                dict(
                    refTb=tr(rn[sl]),
                    extTb=extTb,
                    smallT=np.ascontiguousarray(small),
                    extnatb=extnatb,
                    ident=ident,
                    iotaf=iotaf,
                )
            )
    return in_maps


def finish(sums_stack):
    """sums_stack: [8, 128, 64] per-core {Sa,Sb,N} sums -> scalar loss.

    Per unit u: cols 3u..3u+2 = Sa, Sb, N;  kl = N/Sa - ln Sa + ln Sb;
    smooth-L1(beta=0.5) then branch means (the scalar "all-reduce")."""
    s = np.asarray(sums_stack, dtype=np.float64)
    # col map: d1 f -> 3f; group j (j=0..3) owns 9+12j: d2_j first, then
    # d3_{f,j} at +3+3f; each unit is a {Sa, Sb, N} triplet
    cols_d1 = [3 * f for f in range(NFRAMES)]
    cols_d2 = [9 + 12 * j for j in range(TOPK)]
    cols_d3 = [9 + 12 * j + 3 + 3 * f for f in range(NFRAMES)
               for j in range(TOPK)]

    def hub_of(cols):
        sa = s[..., cols]
        sb = s[..., [c + 1 for c in cols]]
        nn = s[..., [c + 2 for c in cols]]
        kl = nn / sa - np.log(sa) + np.log(sb)
        ax = np.abs(kl)
        return np.where(ax < 0.5, kl * kl, ax - 0.25)

    d1 = hub_of(cols_d1).sum()
    d2 = hub_of(cols_d2).sum()
    d3 = hub_of(cols_d3).sum()
    n_d1 = NFRAMES * B * NUM_REF                 # 3072
    n_d2 = B * NUM_REF * TOPK                    # 4096 (dedup: loop adds 3x)
    n_d3 = NFRAMES * B * NUM_REF * TOPK          # 12288
    return np.float32(d1 / n_d1 + d2 / n_d2 + d3 / n_d3)


def run(in_maps, trace=False):
    nc = get_module()
    res = run_bass_kernel_spmd(nc, in_maps, list(range(8)), trace=trace)
    return res


def kernel(teacher_feats, student_feats, ref_perm, shared_perm):
    in_maps = make_in_maps(teacher_feats, student_feats, ref_perm, shared_perm)
    res = run(in_maps)
    sums = np.stack([r["sums"] for r in res.results])
    return finish(sums)
